# revision 40
# baseline (speedup 1.0000x reference)
"""Trainium2 Bass kernel for nn_MultiHeadBlock (B=4, S=2048, D=512, H=8).

Sharding: 8 cores = 4 batches x 2 query-halves. Each core computes K/V for its
batch's full 2048-key sequence and runs all 8 heads for its 1024 queries.

v2: single fused emission stream. QKV projection matmuls are woven into the
attention units as PE filler so the tensor engine never idles (keeps the HAM
clock-gate warm), scores for the two head-halves are emitted j-major so they
run concurrently on disjoint PE row groups, the residual is folded into W_o
(W_o+I host-side), and LayerNorm's rstd uses a DVE Quake rsqrt so ACT keeps a
single (exp) table set loaded.
"""

import os
import sys

for _p in ("/opt/trn_rl_repo", "/root/.axon_site/_ro/trn_rl_repo"):
    if os.path.isdir(_p) and _p not in sys.path:
        sys.path.insert(0, _p)

import numpy as np

import concourse.bass as bass
import concourse.bacc as bacc
import concourse.mybir as mybir
import concourse.tile as tile
from concourse.masks import make_identity

F32 = mybir.dt.float32
F32R = mybir.dt.float32r
BF16 = mybir.dt.bfloat16
I32 = mybir.dt.int32
ALU = mybir.AluOpType
ACTF = mybir.ActivationFunctionType

B, S, D = 4, 2048, 512
H, DH = 8, 64
SQ = S // 2          # queries per core
NKT = S // 128       # 16 key row-tiles
NDC = D // 128       # 4 contraction chunks
EPS = 1e-5
N_CORES = 8

# key-chunk groups per attention unit: (first kc, count)
# kl=2 so score psum fits: 2 pools x [128,1024] = 4 banks + acc 2 + qkv 2
KG = [(2 * i, 2) for i in range(8)]


def build_program():
    nc = bacc.Bacc("TRN2", target_bir_lowering=False, debug=False,
                   num_devices=N_CORES)

    xt_d = nc.dram_tensor("xt", [D, S], F32R, kind="ExternalInput").ap()
    wqkv_d = nc.dram_tensor("wqkv", [D, 3 * D], F32R, kind="ExternalInput").ap()
    bqkv_d = nc.dram_tensor("bqkv_pt", [128, 12], F32, kind="ExternalInput").ap()
    bvrow_d = nc.dram_tensor("bv_row", [1, D], F32, kind="ExternalInput").ap()
    wo_d = nc.dram_tensor("wo", [D, D], F32R, kind="ExternalInput").ap()
    borow_d = nc.dram_tensor("bo_row", [1, D], F32, kind="ExternalInput").ap()
    gam_d = nc.dram_tensor("gamma_row", [1, D], F32, kind="ExternalInput").ap()
    bet_d = nc.dram_tensor("beta_row", [1, D], F32, kind="ExternalInput").ap()
    maskf_d = nc.dram_tensor("maskf_pt", [128, NKT], F32, kind="ExternalInput").ap()
    out_d = nc.dram_tensor("out", [SQ, D], F32, kind="ExternalOutput").ap()

    with tile.TileContext(nc) as tc:
        with tc.tile_pool(name="const", bufs=1) as cp, \
             tc.tile_pool(name="a_sb", bufs=2) as asb, \
             tc.tile_pool(name="chunk_sb", bufs=5) as csb, \
             tc.tile_pool(name="ps_sE", bufs=1, space="PSUM") as ps_sE, \
             tc.tile_pool(name="ps_sO", bufs=1, space="PSUM") as ps_sO, \
             tc.tile_pool(name="ps_acc", bufs=2, space="PSUM") as ps_acc:
            spool = [ps_sE, ps_sO]

            # ---------- small consts ----------
            bqkv_sb = cp.tile([128, 12], F32, name="bqkv_sb")
            nc.sync.dma_start(out=bqkv_sb[:], in_=bqkv_d)
            maskf_sb = cp.tile([128, NKT], F32, name="maskf_sb")
            nc.sync.dma_start(out=maskf_sb[:], in_=maskf_d)
            rows = cp.tile([1, 4 * D], F32, name="rows")
            nc.sync.dma_start(out=rows[0:1, 0:D], in_=bvrow_d)
            nc.sync.dma_start(out=rows[0:1, D:2 * D], in_=borow_d)
            nc.sync.dma_start(out=rows[0:1, 2 * D:3 * D], in_=gam_d)
            nc.sync.dma_start(out=rows[0:1, 3 * D:4 * D], in_=bet_d)

            # ---------- big input DMAs, ordered for earliest first matmul:
            # per-dc [xt.cb0, wq.q, wq.k], then xt.cb1, wq.v, xt.cb2/3, wo
            xt_sb = [cp.tile([128, S], F32R, name=f"xt{dc}")
                     for dc in range(NDC)]
            wq_sb = [cp.tile([128, 3 * D], F32R, name=f"wq{dc}")
                     for dc in range(NDC)]
            wo_sb = [cp.tile([128, D], F32R, name=f"wo{c}") for c in range(NDC)]

            def dma_xt(dc, cb):
                nc.sync.dma_start(
                    out=xt_sb[dc][:, cb * 512:(cb + 1) * 512],
                    in_=xt_d[dc * 128:(dc + 1) * 128, cb * 512:(cb + 1) * 512])

            def dma_wq(dc, cb):
                nc.sync.dma_start(
                    out=wq_sb[dc][:, cb * 512:(cb + 1) * 512],
                    in_=wqkv_d[dc * 128:(dc + 1) * 128,
                               cb * 512:(cb + 1) * 512])

            for dc in range(NDC):
                dma_xt(dc, 0)
                dma_wq(dc, 0)
                dma_wq(dc, 1)
            for dc in range(NDC):
                dma_xt(dc, 1)
            for dc in range(NDC):
                dma_wq(dc, 2)
            for dc in range(NDC):
                dma_xt(dc, 2)
                dma_xt(dc, 3)
            for c in range(NDC):
                nc.sync.dma_start(out=wo_sb[c][:],
                                  in_=wo_d[c * 128:(c + 1) * 128, :])

            # ---------- warmup, broadcast rows ----------
            # NOTE: warmup matmuls must use NONZERO data — an all-zeros
            # warmup toggles no MAC activity and the HAM clock-gate never
            # un-throttles (measured +40us)
            ident_f = cp.tile([128, 128], F32, name="ident_f")
            make_identity(nc, ident_f[:])
            ident = cp.tile([128, 128], F32R, name="ident")
            nc.vector.tensor_copy(ident[:], ident_f[:])
            # 80 warmup MMs (~8.6us) span two full 3.4us HAM windows, so one
            # is guaranteed fully busy regardless of window phase — 40 MMs
            # (4.3us) straddled the boundary and left the ramp cold to ~37us
            with tc.tile_pool(name="warm", bufs=1, space="PSUM") as warmp:
                wps = warmp.tile([128, 128], F32, name="wps")
                for _ in range(80):
                    nc.tensor.matmul(wps[:], lhsT=ident[:], rhs=ident[:],
                                     start=True, stop=True)
            ones8 = cp.tile([128, 8], F32, name="ones8")
            nc.vector.memset(ones8[:], 1.0)

            bv_bc = cp.tile([128, D], F32, name="bv_bc")
            bo_bc = cp.tile([128, D], F32, name="bo_bc")
            gam_bc = cp.tile([128, D], F32, name="gam_bc")
            bet_bc = cp.tile([128, D], F32, name="bet_bc")
            for j, t in enumerate((bv_bc, bo_bc, gam_bc, bet_bc)):
                nc.gpsimd.partition_broadcast(
                    t[:], rows[0:1, j * D:(j + 1) * D], channels=128)

            # ---------- persistent activation tiles ----------
            q_t = [cp.tile([128, SQ], F32R, name=f"qt{t}") for t in range(4)]
            k_t = [cp.tile([128, S], F32R, name=f"kt{t}") for t in range(4)]
            # v_aug and the exp(scores) tiles are bf16: halves SBUF, full
            # matmul rate, and the softmax ratio cancels most rounding
            v_aug = [cp.tile([128, H * (DH + 1)], BF16, name=f"va{t}")
                     for t in range(NKT)]
            sumx8 = cp.tile([128, 8], F32, name="sumx8")
            sumsq8 = cp.tile([128, 8], F32, name="sumsq8")

            # ---------- QKV emit helpers (4 accumulation MMs each) ----------
            with tc.tile_pool(name="p1v", bufs=2) as p1v, \
                 tc.tile_pool(name="p1ps", bufs=2, space="PSUM") as p1ps:

                def emit_q(pair, qh):
                    ps = p1ps.tile([128, 512], F32, tag="qkv",
                                   name=f"psq{pair}_{qh}")
                    for dc in range(NDC):
                        nc.tensor.matmul(
                            ps[:],
                            lhsT=wq_sb[dc][:, pair * 128:(pair + 1) * 128],
                            rhs=xt_sb[dc][:, qh * 512:(qh + 1) * 512],
                            start=(dc == 0), stop=(dc == NDC - 1))
                    nc.vector.tensor_scalar_add(
                        out=q_t[pair][:, qh * 512:(qh + 1) * 512],
                        in0=ps[:], scalar1=bqkv_sb[:, pair:pair + 1])

                def emit_k(pair, kq):
                    ps = p1ps.tile([128, 512], F32, tag="qkv",
                                   name=f"psk{pair}_{kq}")
                    for dc in range(NDC):
                        nc.tensor.matmul(
                            ps[:],
                            lhsT=wq_sb[dc][:, D + pair * 128:
                                           D + (pair + 1) * 128],
                            rhs=xt_sb[dc][:, kq * 512:(kq + 1) * 512],
                            start=(dc == 0), stop=(dc == NDC - 1))
                    nc.vector.tensor_scalar_add(
                        out=k_t[pair][:, kq * 512:(kq + 1) * 512],
                        in0=ps[:], scalar1=bqkv_sb[:, 4 + pair:5 + pair])

                def emit_v(rt):
                    ps = p1ps.tile([128, 512], F32, tag="qkv",
                                   name=f"psv{rt}")
                    for dc in range(NDC):
                        nc.tensor.matmul(
                            ps[:],
                            lhsT=xt_sb[dc][:, rt * 128:(rt + 1) * 128],
                            rhs=wq_sb[dc][:, 2 * D:3 * D],
                            start=(dc == 0), stop=(dc == NDC - 1))
                    vtmp = p1v.tile([128, 512], F32, tag="vtmp",
                                    name=f"vtmp{rt}")
                    nc.vector.tensor_add(vtmp[:], ps[:], bv_bc[:])
                    va_v = v_aug[rt][:, :].rearrange(
                        "p (h c) -> p h c", c=DH + 1)[:, :, 0:DH]
                    vt_v = vtmp[:, :].rearrange("p (h c) -> p h c", c=DH)
                    nc.vector.tensor_scalar_mul(
                        out=va_v, in0=vt_v, scalar1=maskf_sb[:, rt:rt + 1])
                    va_one = v_aug[rt][:, :].rearrange(
                        "p (h c) -> p h c", c=DH + 1)[:, :, DH:DH + 1]
                    on_v = ones8[:, :].rearrange("p (h c) -> p h c", c=1)
                    nc.vector.tensor_scalar_mul(
                        out=va_one, in0=on_v, scalar1=maskf_sb[:, rt:rt + 1])

                # ---------- attention emit helpers ----------
                def emit_scores(qt, pair, k0, kl):
                    # j-major, e-minor: consecutive MMs target disjoint PE
                    # row groups (rows 0-63 / 64-127) and run concurrently
                    scs = [spool[e].tile([128, kl * 512], F32, tag=f"sc{e}",
                                         name=f"sc{e}_{qt}_{pair}_{k0}")
                           for e in range(2)]
                    for j in range(kl):
                        kc = k0 + j
                        for e in range(2):
                            off = 64 * e
                            nc.tensor.matmul(
                                scs[e][:, j * 512:(j + 1) * 512],
                                lhsT=k_t[pair][off:off + 64,
                                               kc * 128:(kc + 1) * 128],
                                rhs=q_t[pair][off:off + 64,
                                              qt * 512:(qt + 1) * 512],
                                start=True, stop=True)
                    return scs

                def emit_exp(qt, pair, k0, kl, scs):
                    ats = []
                    for e in range(2):
                        at = asb.tile([128, kl * 512], BF16, tag=f"at{e}",
                                      bufs=2, name=f"at{e}_{qt}_{pair}_{k0}")
                        nc.scalar.activation(at[:], scs[e][:], ACTF.Exp)
                        ats.append(at)
                    return ats

                def emit_av(pair, k0, kl, ats, accs):
                    for j in range(kl):
                        kc = k0 + j
                        for e in range(2):
                            h = 2 * pair + e
                            nc.tensor.matmul(
                                accs[e][:],
                                lhsT=v_aug[kc][:, h * (DH + 1):
                                               (h + 1) * (DH + 1)],
                                rhs=ats[e][:, j * 512:(j + 1) * 512],
                                start=(kc == 0), stop=(kc == NKT - 1))

                def emit_norm(qt, pair, accs):
                    # per-head 1/sumexp normalization -> chunk [128, 512].
                    # NOTE: copy accs out of PSUM first — reading the acc
                    # tile directly in the final multiply holds its PSUM
                    # buffer through the recip chain and stalls the pool
                    # rotation at every unit boundary (measured +20us)
                    chunk = csb.tile([128, 512], F32R, tag="chunk",
                                     name=f"chunk{qt}_{pair}")
                    for e in range(2):
                        au = asb.tile([128, 512], F32, tag="attu", bufs=2,
                                      name=f"au{qt}_{pair}_{e}")
                        nc.vector.tensor_copy(au[0:DH + 1, :], accs[e][:])
                        # sumexp row copied to partition 0 first:
                        # partition_broadcast needs a partition-0 source on HW
                        se0 = asb.tile([1, 512], F32, tag="se0",
                                       name=f"se0_{qt}_{pair}_{e}")
                        nc.vector.tensor_copy(se0[0:1, :], au[64:65, :])
                        rp = asb.tile([64, 512], F32, tag="rpre",
                                      name=f"rp{qt}_{pair}_{e}")
                        nc.gpsimd.partition_broadcast(
                            rp[:], se0[0:1, :], channels=64)
                        rs = asb.tile([64, 512], F32, tag="rsb",
                                      name=f"rs{qt}_{pair}_{e}")
                        scr = asb.tile([64, 512], F32, tag="scr512",
                                       name=f"scr{qt}_{pair}_{e}")
                        nc.vector.reciprocal_approx_accurate(
                            out=rs[:], in_=rp[:], scratch=scr[:])
                        nc.vector.tensor_mul(
                            chunk[64 * e:64 * (e + 1), :], au[0:64, :], rs[:])
                    return chunk

                def emit_ln1(i, xs):
                    # LayerNorm for one 128-query block; rstd via DVE Quake
                    # rsqrt, fused to few ops (ACT keeps its exp table set)
                    mu = asb.tile([128, 1], F32, tag="mu1", bufs=4,
                                  name=f"mu{i}")
                    var = asb.tile([128, 1], F32, tag="var1", bufs=4,
                                   name=f"var{i}")
                    tmp = asb.tile([128, 1], F32, tag="tmp1", bufs=4,
                                   name=f"tmp{i}")
                    nc.vector.tensor_scalar_mul(
                        out=mu[:], in0=sumx8[:, i:i + 1], scalar1=1.0 / D)
                    # tmp = mu^2 - eps;  var = sumsq/D - tmp = var_raw + eps
                    nc.vector.tensor_scalar(
                        out=tmp[:], in0=mu[:], scalar1=mu[:, 0:1],
                        scalar2=EPS, op0=ALU.mult, op1=ALU.subtract)
                    nc.vector.tensor_scalar(
                        out=var[:], in0=sumsq8[:, i:i + 1], scalar1=1.0 / D,
                        scalar2=tmp[:, 0:1], op0=ALU.mult, op1=ALU.subtract)
                    seed = asb.tile([128, 1], I32, tag="seed1", bufs=4,
                                    name=f"sd{i}")
                    nc.vector.tensor_scalar(
                        out=seed[:], in0=var[:].bitcast(I32),
                        scalar1=1, scalar2=None, op0=ALU.arith_shift_right)
                    nc.vector.tensor_scalar(
                        out=seed[:], in0=seed[:], scalar1=-1,
                        scalar2=0x5F3759DF, op0=ALU.mult, op1=ALU.add)
                    rstd = seed[:].bitcast(F32)
                    for _ in range(2):
                        # t = rstd^2;  t = (t*var)*-0.5;  rstd = (1.5+t)*rstd
                        nc.vector.tensor_scalar(
                            out=tmp[:], in0=rstd, scalar1=seed[:, 0:1].bitcast(F32),
                            scalar2=var[:, 0:1], op0=ALU.mult, op1=ALU.mult)
                        nc.vector.tensor_scalar(
                            out=tmp[:], in0=tmp[:], scalar1=-0.5,
                            scalar2=1.5, op0=ALU.mult, op1=ALU.add)
                        nc.vector.tensor_mul(seed[:].bitcast(F32), rstd,
                                             tmp[:])
                    y = asb.tile([128, D], F32, tag="y", name=f"y{i}")
                    nc.vector.tensor_scalar(
                        out=y[:], in0=xs[:], scalar1=mu[:, 0:1],
                        scalar2=seed[:, 0:1].bitcast(F32),
                        op0=ALU.subtract, op1=ALU.mult)
                    nc.vector.tensor_mul(y[:], y[:], gam_bc[:])
                    nc.vector.tensor_add(y[:], y[:], bet_bc[:])
                    nc.sync.dma_start(out=out_d[i * 128:(i + 1) * 128, :],
                                      in_=y[:])

                def emit_oproj(qt, chunks):
                    # fused O-proj + per-qsub LayerNorm: each 128-query block
                    # normalizes and streams out as soon as its stats land
                    for qsub in range(4):
                        i = qt * 4 + qsub
                        po = ps_acc.tile([128, 512], F32, tag="acc",
                                         name=f"po{i}")
                        for c in range(NDC):
                            nc.tensor.matmul(
                                po[:],
                                lhsT=chunks[c][:, qsub * 128:(qsub + 1) * 128],
                                rhs=wo_sb[c][:],
                                start=(c == 0), stop=(c == NDC - 1))
                        # residual folded into W_o host-side (wo := W_o + I)
                        xs = asb.tile([128, D], F32, tag="xsb", bufs=4,
                                      name=f"x{i}")
                        nc.vector.scalar_tensor_tensor(
                            out=xs[:], in0=po[:], scalar=0.0,
                            in1=bo_bc[:], op0=ALU.add, op1=ALU.add,
                            accum_out=sumx8[:, i:i + 1])
                        sq = asb.tile([128, 512], F32, tag="scr512",
                                      name=f"sq{i}")
                        nc.vector.scalar_tensor_tensor(
                            out=sq[:], in0=xs[:], scalar=0.0,
                            in1=xs[:], op0=ALU.add, op1=ALU.mult,
                            accum_out=sumsq8[:, i:i + 1])
                        emit_ln1(i, xs)

                # ---------- filler schedule ----------
                # units in qt-major order; filler[u][g] = list of emit thunks
                # run right after that unit's group-g scores
                filler = {u: {g: [] for g in range(len(KG) + 1)}
                          for u in range(8)}

                def F(u, g, fn, *a):
                    filler[u][g].append((fn, a))

                # unit 0 carries V-proj JIT (AV(g) needs v_aug[kc in g];
                # emitted right after scores(g) so it lands one group early)
                for g, (k0, kl) in enumerate(KG):
                    for rt in range(k0, k0 + kl):
                        F(0, g, emit_v, rt)
                # K kq1-3 of pair p JIT inside unit (0,p): group g reads
                # key block kc=2g..2g+1, i.e. kq=g//2, so kq1 must land
                # before g2, kq2 before g4, kq3 before g6
                for p in range(4):
                    F(p, 0, emit_k, p, 1)
                    F(p, 2, emit_k, p, 2)
                    F(p, 4, emit_k, p, 3)
                    if p < 3:
                        # next pair's first-half Q and kq0 late in this unit
                        F(p, 6, emit_q, p + 1, 0)
                        F(p, 7, emit_k, p + 1, 0)
                # second-half queries: needed from unit 4 on
                F(3, 1, emit_q, 0, 1)
                F(3, 3, emit_q, 1, 1)
                F(4, 1, emit_q, 2, 1)
                F(5, 1, emit_q, 3, 1)

                # ---------- main schedule ----------
                emit_q(0, 0)
                emit_k(0, 0)
                chunks_by_qt = {0: [], 1: []}
                for u in range(8):
                    qt, pair = divmod(u, 4)
                    accs = [ps_acc.tile([DH + 1, 512], F32, tag="acc",
                                        name=f"acc{qt}_{pair}_{e}")
                            for e in range(2)]
                    prev = None
                    for g, (k0, kl) in enumerate(KG):
                        scs = emit_scores(qt, pair, k0, kl)
                        for fn, a in filler[u][g]:
                            fn(*a)
                        if prev is not None:
                            emit_av(pair, prev[0], prev[1], prev[2], accs)
                        ats = emit_exp(qt, pair, k0, kl, scs)
                        prev = (k0, kl, ats)
                    for fn, a in filler[u][len(KG)]:
                        fn(*a)
                    emit_av(pair, prev[0], prev[1], prev[2], accs)
                    chunks_by_qt[qt].append(emit_norm(qt, pair, accs))
                    if u == 3:
                        emit_oproj(0, chunks_by_qt[0])
                emit_oproj(1, chunks_by_qt[1])
    nc.compile()
    return nc


_CACHED = {}


def _get_program():
    if "nc" not in _CACHED:
        _CACHED["nc"] = build_program()
    return _CACHED["nc"]


def make_in_maps(inputs, mask, W_qkv, b_qkv, W_o, b_o, gamma, beta):
    inputs = np.asarray(inputs, np.float32)
    mask = np.asarray(mask)
    W_qkv = np.asarray(W_qkv, np.float32)
    b_qkv = np.asarray(b_qkv, np.float32)
    W_o = np.asarray(W_o, np.float32)
    b_o = np.asarray(b_o, np.float32)
    gamma = np.asarray(gamma, np.float32)
    beta = np.asarray(beta, np.float32)

    shared = {
        "wqkv": np.ascontiguousarray(W_qkv),
        "bqkv_pt": np.ascontiguousarray(b_qkv.reshape(12, 128).T),
        "bv_row": np.ascontiguousarray(b_qkv[2 * D:3 * D].reshape(1, D)),
        "wo": np.ascontiguousarray(W_o + np.eye(D, dtype=np.float32)),
        "bo_row": np.ascontiguousarray(b_o.reshape(1, D)),
        "gamma_row": np.ascontiguousarray(gamma.reshape(1, D)),
        "beta_row": np.ascontiguousarray(beta.reshape(1, D)),
    }
    in_maps = []
    for c in range(N_CORES):
        b, half = divmod(c, 2)
        xb = inputs[b]
        mk = mask[b].astype(np.float32)
        if half:
            order = np.r_[SQ:S, 0:SQ]
            xb = xb[order]
            mk = mk[order]
        m = dict(shared)
        m["xt"] = np.ascontiguousarray(xb.T)
        m["maskf_pt"] = np.ascontiguousarray(mk.reshape(NKT, 128).T)
        in_maps.append(m)
    return in_maps


def kernel(inputs, mask, W_qkv, b_qkv, W_o, b_o, gamma, beta):
    from concourse.bass_utils import run_bass_kernel_spmd

    nc = _get_program()
    in_maps = make_in_maps(inputs, mask, W_qkv, b_qkv, W_o, b_o, gamma, beta)
    res = run_bass_kernel_spmd(nc, in_maps, list(range(N_CORES)))
    out = np.empty((B, S, D), np.float32)
    for c in range(N_CORES):
        b, half = divmod(c, 2)
        out[b, half * SQ:(half + 1) * SQ, :] = res.results[c]["out"]
    return out
